# revision 14
# baseline (speedup 1.0000x reference)
"""BERT layer (B=8, S=512, H=768, NH=12, DH=64, FF=3072) on 8 Trainium2 cores.

Data-parallel over batch (1 element/core, no collectives), feature-major
on-chip layout (activations as X^T [feat partitions, tokens free]), and
fp8e4m3 DoubleRow matmuls (2 k-planes per instruction, 0.5 PE cycles/row)
for every large GEMM:

  Q^T/K^T : fp8-DR (Wq,Wk x64 fp8; x fp8) -> bf16 via DVE descale(+bias)
  V       : fp8-DR (token-major), DVE descale -> v_q fp8
  S^T     : bf16 (K on partitions, per-head, quadrant-packed)
  P^T     : exp on ACT (scale=1/8 folded, mask bias) -> fp8
  den     : fp8-DR ones(=1/64) @ P^T -> broadcast rows; recip -> x64 ctx scale
  ctx^T   : fp8-DR (v_q @ P^T), DVE normalize -> ctx fp8 (x64 scaled)
  attn    : fp8-DR (Wo x64), descale 1/(64*64) + residual via DVE stt
            (bo + bv@Wo folded into the residual copy of x on host)
  LN1     : stats via f32r ones-matmuls; gamma/beta folded into W1/b1 (host),
            so FFN1 consumes raw (r1-mu)*rstd as fp8
  FFN1    : fp8-DR -> gelu on ACT (scale 1/64, bias b1') -> fp8
  FFN2    : fp8-DR -> descale + (b2+beta1) via DVE tensor_scalar,
            r2 = gamma1*nrm1 + ff via DVE stt
  LN2     : stats matmuls; output = gamma2*nrm2 + beta2 via tensor_scalar
"""

from contextlib import ExitStack

import numpy as np
import ml_dtypes

import concourse.bass as bass
from concourse import bacc
import concourse.tile as tile
from concourse import mybir
from concourse.bass_utils import run_bass_kernel_spmd

F32 = mybir.dt.float32
F32R = mybir.dt.float32r
BF16 = mybir.dt.bfloat16
FP8 = mybir.dt.float8e4
AF = mybir.ActivationFunctionType
ALU = mybir.AluOpType
DR = mybir.MatmulPerfMode.DoubleRow

B, S, H, NH, DH, FF = 8, 512, 768, 12, 64, 3072
EPS = 1e-3
CH = H // 128    # 6 hidden chunks
CU = CH // 2     # 3 DR chunk-pairs
CF = FF // 128   # 24 ff chunks
FU = CF // 2     # 12 DR pairs
T = S // 128     # 4 token/key chunks
NP = NH // 2     # 6 head pairs
SW = 64.0        # fp8 weight scale
RS = 1.0 / SW

# consts tile column map: [128, NCONST]
BQ, BK, G1, B2E, G2, B2O = 0, 6, 12, 18, 24, 30
B1 = 36          # 24 cols (b1 + W1^T beta1)
MB = B1 + CF     # 4 cols of mask bias
NCONST = MB + T


def ts(i, n):
    return slice(i * n, (i + 1) * n)


def build_nc(gelu_mode="hw", repeats=1, simple=True):
    nc = bacc.Bacc("TRN2", target_bir_lowering=False, debug=False)

    xq_d = nc.declare_dram_parameter("xq", [H, S], FP8, isOutput=False)
    xr_d = nc.declare_dram_parameter("xr", [H, S], F32, isOutput=False)
    wqb_d = nc.declare_dram_parameter("wqb", [CH, 128, CH, 128], FP8,
                                      isOutput=False)
    wkb_d = nc.declare_dram_parameter("wkb", [CH, 128, CH, 128], FP8,
                                      isOutput=False)
    wv_d = nc.declare_dram_parameter("wv", [H, H], FP8, isOutput=False)
    wo_d = nc.declare_dram_parameter("wo", [H, H], FP8, isOutput=False)
    w1_d = nc.declare_dram_parameter("w1b", [CF, 128, 2 * CH, 128], FP8,
                                     isOutput=False)
    w2_d = nc.declare_dram_parameter("w2b", [CH, 128, 2 * CF, 128], FP8,
                                     isOutput=False)
    c_d = nc.declare_dram_parameter("consts", [128, NCONST], F32, isOutput=False)
    out_d = nc.declare_dram_parameter("outT", [H, S], F32, isOutput=True)

    # feature-major DRAM views: (p, i, n) = W[i*128+p, n]
    def fmaj(d):
        return d.rearrange("(i p) n -> p i n", p=128)

    def layer_norm(tc, nc, pspool, tmp, ones_sum, eps_sb, src, apply_out):
        """Normalize src [128, CH, S] over features, in two token-halves so
        the serial stats->rstd chain of half 0 overlaps half 1's matmuls.

        Calls apply_out(jj, cen, rstd, sl) per (chunk, token-half) — caller
        fuses the final (cen * rstd) multiply with whatever comes next.
        """
        HS = S // 2
        sum_ps = pspool.tile([128, S], F32, tag="lnsum", bufs=1)
        sq_ps = pspool.tile([128, S], F32, tag="lnsq", bufs=1)
        sqt = []
        for i in range(CH):
            sq = tmp.tile([128, S], F32R, tag="sq", bufs=3, name="sq")
            nc.scalar.activation(out=sq, in_=src[:, i, :], func=AF.Square)
            sqt.append(sq)
        for h in range(2):
            sl = ts(h, HS)
            for i in range(CH):
                nc.tensor.matmul(
                    sum_ps[:, sl], ones_sum[:, :], src[:, i, sl],
                    start=(i == 0), stop=(i == CH - 1),
                )
            for i in range(CH):
                nc.tensor.matmul(
                    sq_ps[:, sl], ones_sum[:, :], sqt[i][:, sl],
                    start=(i == 0), stop=(i == CH - 1),
                )
            mean = tmp.tile([128, S], F32, tag="mean", bufs=1, name="mean")
            nc.vector.tensor_scalar_mul(mean[:, sl], sum_ps[:, sl], 1.0 / H)
            negm2 = tmp.tile([128, HS], F32, tag="negm2", bufs=2, name="negm2")
            nc.vector.scalar_tensor_tensor(
                out=negm2, in0=mean[:, sl], scalar=-1.0, in1=mean[:, sl],
                op0=ALU.mult, op1=ALU.mult,
            )
            var = tmp.tile([128, HS], F32, tag="var", bufs=2, name="var")
            nc.vector.scalar_tensor_tensor(
                out=var, in0=sq_ps[:, sl], scalar=1.0 / H, in1=negm2,
                op0=ALU.mult, op1=ALU.add,
            )
            sd = tmp.tile([128, HS], F32, tag="sd", bufs=2, name="sd")
            nc.scalar.activation(out=sd, in_=var, func=AF.Sqrt, bias=eps_sb[:, :])
            rstd = tmp.tile([128, S], F32, tag="rstd", bufs=1, name="rstd")
            nc.vector.reciprocal_approx_fast(out=rstd[:, sl], in_=sd)
            for jj in range(CH):
                cen = tmp.tile([128, S], F32, tag="cen", bufs=4, name="cen")
                nc.vector.tensor_tensor(
                    out=cen[:, sl], in0=src[:, jj, sl], in1=mean[:, sl],
                    op=ALU.subtract,
                )
                apply_out(jj, cen, rstd, sl)

    with tile.TileContext(nc) as tc, ExitStack() as top:
        cpool = top.enter_context(tc.tile_pool(name="cpool", bufs=1))
        c_sb = cpool.tile([128, NCONST], F32, name="c_sb")
        nc.sync.dma_start(out=c_sb, in_=c_d[:, :])
        ones8 = cpool.tile([128, 2, 64], FP8, name="ones8")
        nc.vector.memset(ones8, RS)  # 1/64: folds the x64 ctx scale into den
        ones_f32 = cpool.tile([128, 128], F32, name="ones_f32")
        nc.vector.memset(ones_f32, 1.0)
        ones_sum = cpool.tile([128, 128], F32R, name="ones_sum")
        nc.vector.tensor_copy(out=ones_sum, in_=ones_f32)
        eps_sb = cpool.tile([128, 1], F32, name="eps_sb")
        nc.vector.memset(eps_sb, EPS)

        mid = top.enter_context(tc.tile_pool(name="mid", bufs=1))
        tmp = top.enter_context(tc.tile_pool(name="tmp", bufs=1))

        for _rep in range(repeats):
            with ExitStack() as s_ac:
                apool = s_ac.enter_context(tc.tile_pool(name="apool", bufs=1))
                xq = apool.tile([128, CH, S], FP8, name="xq")
                for _i in range(CH):
                    nc.sync.dma_start(out=xq[:, _i, :], in_=fmaj(xq_d)[:, _i, :])
                xr = apool.tile([128, CH, S], F32, name="xr")
                for _i in range(CH):
                    nc.sync.dma_start(out=xr[:, _i, :], in_=fmaj(xr_d)[:, _i, :])
                qT = apool.tile([128, CH, S], BF16, name="qT")
                kT = apool.tile([128, CH, S], BF16, name="kT")
                v_q = apool.tile([128, T, NH, DH], FP8, name="v_q")
                ctxq = apool.tile([128, CH, S], FP8, name="ctxq")

                wopool = s_ac.enter_context(tc.tile_pool(name="wopool", bufs=1))
                wo_sb = wopool.tile([128, CH, H], FP8, name="wo_sb")

                # ---- Fused phase A+B: V first, then per head pair:
                # project Q_j/K_j -> S^T -> exp -> denom -> ctx.
                with ExitStack() as s_a:
                    wqkv = s_a.enter_context(tc.tile_pool(name="wqkv", bufs=1))
                    wv_sb = wqkv.tile([128, CH, H], FP8, name="wv_sb")
                    for _i in range(CH):
                        nc.sync.dma_start(
                            out=wv_sb[:, _i, :], in_=fmaj(wv_d)[:, _i, :]
                        )
                    wqkpool = s_a.enter_context(
                        tc.tile_pool(name="wqkpool", bufs=3)
                    )
                    psA = s_a.enter_context(
                        tc.tile_pool(name="psA", bufs=1, space="PSUM")
                    )
                    bpool = s_a.enter_context(tc.tile_pool(name="bpool", bufs=1))

                    def project_qk(j):
                        for blk_d, dest, bcol in ((wqb_d, qT, BQ), (wkb_d, kT, BK)):
                            wt = wqkpool.tile([128, CH, 128], FP8, tag="wqk",
                                              name="wt")
                            nc.sync.dma_start(out=wt, in_=blk_d[j])
                            ps = psA.tile([128, S], F32, tag="mm", bufs=2,
                                          name="psqk")
                            for u in range(CU):
                                nc.tensor.matmul(
                                    ps[:, :],
                                    wt[:, ts(u, 2), :],
                                    xq[:, ts(u, 2), :],
                                    start=(u == 0),
                                    stop=(u == CU - 1),
                                    perf_mode=DR,
                                )
                            # (ps * 1/SW) + bias  -> bf16
                            if simple:
                                nc.vector.tensor_scalar_mul(
                                    dest[:, j, :], ps[:, :], RS
                                )
                            else:
                                nc.vector.tensor_scalar(
                                    out=dest[:, j, :], in0=ps[:, :],
                                    scalar1=RS,
                                    scalar2=c_sb[:, bcol + j: bcol + j + 1],
                                    op0=ALU.mult, op1=ALU.add,
                                )

                    project_qk(0)
                    for t in range(T):
                        for half in range(2):
                            psf = psA.tile([128, S], F32, tag="mm", bufs=2,
                                           name="psv")
                            ps = psf[:, 0:384]
                            for u in range(CU):
                                nc.tensor.matmul(
                                    ps[:, :],
                                    xq[:, ts(u, 2), ts(t, 128)],
                                    wv_sb[:, ts(u, 2), ts(half, 384)],
                                    start=(u == 0),
                                    stop=(u == CU - 1),
                                    perf_mode=DR,
                                )
                            nc.vector.tensor_scalar_mul(
                                v_q[:, t, ts(half, 6), :].rearrange(
                                    "p h d -> p (h d)"
                                ),
                                ps[:, :],
                                RS,
                            )

                    for j in range(NP):
                        if j > 0:
                            project_qk(j)
                        if j == 2:
                            for _i in range(CH):
                                nc.sync.dma_start(
                                    out=wo_sb[:, _i, :], in_=fmaj(wo_d)[:, _i, :]
                                )
                        es = bpool.tile([128, T, S], FP8, tag="es", bufs=2,
                                        name="es")
                        esb = bpool.tile([128, T, S], FP8, tag="esb", bufs=2,
                                         name="esb")
                        for t in range(T):
                            for half, e in ((0, es), (1, esb)):
                                ps = psA.tile([128, S], F32, tag="sT", bufs=2,
                                              name="ps_sT")
                                nc.tensor.matmul(
                                    ps[:, :],
                                    kT[ts(half, 64), j, ts(t, 128)],
                                    qT[ts(half, 64), j, :],
                                    start=True,
                                    stop=True,
                                    tile_position=(half * 64, 0),
                                )
                                nc.scalar.activation(
                                    out=e[:, t, :],
                                    in_=ps[:, :],
                                    func=AF.Exp,
                                    bias=(0.0 if simple
                                          else c_sb[:, MB + t: MB + t + 1]),
                                    scale=0.125,
                                )
                        # per half: denominator rows (x 1/64), recip, ctx^T,
                        # normalize into ctxq (DR matmuls need dst base 0;
                        # the half-b partition shift happens in the DVE write)
                        for half, e in ((0, es), (1, esb)):
                            den_ps = psA.tile([128, S], F32, tag="den",
                                              bufs=1, name="den_ps")
                            for u in range(T // 2):
                                nc.tensor.matmul(
                                    den_ps[0:64, :],
                                    ones8[:, :, :],
                                    e[:, ts(u, 2), :],
                                    start=(u == 0),
                                    stop=(u == T // 2 - 1),
                                    perf_mode=DR,
                                )
                            recip = bpool.tile([128, S], F32, tag="recip",
                                               bufs=2, name="recip")
                            nc.vector.reciprocal_approx_fast(
                                out=recip[0:64, :], in_=den_ps[0:64, :]
                            )
                            ctx_ps = psA.tile([128, S], F32, tag="ctx",
                                              bufs=2, name="ctx_ps")
                            for u in range(T // 2):
                                nc.tensor.matmul(
                                    ctx_ps[0:64, :],
                                    v_q[:, ts(u, 2), 2 * j + half, :],
                                    e[:, ts(u, 2), :],
                                    start=(u == 0),
                                    stop=(u == T // 2 - 1),
                                    perf_mode=DR,
                                )
                            nc.vector.tensor_tensor(
                                out=ctxq[ts(half, 64), j, :],
                                in0=ctx_ps[0:64, :],
                                in1=recip[0:64, :],
                                op=ALU.mult,
                            )

                # ---- Phase C: Wo^T @ ctx^T (fp8-DR), +residual (xr), LN1 ----
                r1T = mid.tile([128, CH, S], F32R, name="r1T")
                nrm1q = mid.tile([128, CH, S], FP8, name="nrm1q")
                nrm1f = mid.tile([128, CH, S], F32, name="nrm1f")
                with ExitStack() as s_c:
                    psC = s_c.enter_context(
                        tc.tile_pool(name="psC", bufs=1, space="PSUM")
                    )
                    for j in range(CH):
                        ps = psC.tile([128, S], F32, tag="mm", bufs=3, name="ps_wo")
                        for u in range(CU):
                            nc.tensor.matmul(
                                ps[:, :],
                                wo_sb[:, ts(u, 2), ts(j, 128)],
                                ctxq[:, ts(u, 2), :],
                                start=(u == 0),
                                stop=(u == CU - 1),
                                perf_mode=DR,
                            )
                        # r1 = ps/(SW*SW) + xr   (bo+bv@Wo folded into xr)
                        nc.vector.scalar_tensor_tensor(
                            out=r1T[:, j, :],
                            in0=ps[:, :],
                            scalar=RS * RS,
                            in1=xr[:, j, :],
                            op0=ALU.mult,
                            op1=ALU.add,
                        )

                    def apply_ln1(jj, cen, rstd, sl):
                        # gamma1/beta1 folded into W1/b1; keep raw nrm (f32)
                        # for the r2 residual and fp8 for FFN1.
                        nc.vector.tensor_tensor(
                            out=nrm1f[:, jj, sl], in0=cen[:, sl],
                            in1=rstd[:, sl], op=ALU.mult,
                        )
                        nc.vector.tensor_copy(
                            out=nrm1q[:, jj, sl], in_=nrm1f[:, jj, sl]
                        )

                    layer_norm(tc, nc, psC, tmp, ones_sum, eps_sb, r1T, apply_ln1)

            # ---- Phase D: ff1 = gelu(W1'^T @ nrm1 + b1'), fp8-DR ----
            with ExitStack() as s_de:
                fpool = s_de.enter_context(tc.tile_pool(name="fpool", bufs=1))
                geluT = fpool.tile([128, CF, S], FP8, name="geluT")
                w1pool = s_de.enter_context(tc.tile_pool(name="w1pool", bufs=12))
                psD = s_de.enter_context(tc.tile_pool(name="psD", bufs=1,
                                                      space="PSUM"))
                for f in range(CF):
                    w1t = w1pool.tile([128, 2 * CH, 128], FP8, tag="w1",
                                      name="w1t")
                    nc.sync.dma_start(out=w1t, in_=w1_d[f])
                    ps = psD.tile([128, S], F32, tag="mm", bufs=4, name="ps_f1")
                    for u in range(2 * CU):  # hi then lo weight planes
                        nc.tensor.matmul(
                            ps[:, :],
                            w1t[:, ts(u, 2), :],
                            nrm1q[:, ts(u % CU, 2), :],
                            start=(u == 0),
                            stop=(u == 2 * CU - 1),
                            perf_mode=DR,
                        )
                    nc.scalar.activation(
                        out=geluT[:, f, :],
                        in_=ps[:, :],
                        func=AF.Gelu,
                        bias=(0.0 if simple else c_sb[:, B1 + f: B1 + f + 1]),
                        scale=RS,
                    )

                # ---- Phase E: ff2 (fp8-DR), r2 = g1*nrm1 + ff + (b2+beta1),
                # LN2 ----
                r2T = mid.tile([128, CH, S], F32R, name="r2T")
                w2pool = s_de.enter_context(tc.tile_pool(name="w2pool", bufs=6))
                for j in range(CH):
                    w2t = w2pool.tile([128, 2 * CF, 128], FP8, tag="w2",
                                      name="w2t")
                    nc.sync.dma_start(out=w2t, in_=w2_d[j])
                    ps = psD.tile([128, S], F32, tag="mm", bufs=4, name="ps_f2")
                    for u in range(2 * FU):  # hi then lo weight planes
                        nc.tensor.matmul(
                            ps[:, :],
                            w2t[:, ts(u, 2), :],
                            geluT[:, ts(u % FU, 2), :],
                            start=(u == 0),
                            stop=(u == 2 * FU - 1),
                            perf_mode=DR,
                        )
                    fft = tmp.tile([128, S], F32, tag="fft", bufs=2, name="fft")
                    if simple:
                        # r2 = ps/SW + nrm1 (gamma1==1, b2+beta1==0)
                        nc.vector.scalar_tensor_tensor(
                            out=r2T[:, j, :],
                            in0=ps[:, :],
                            scalar=RS,
                            in1=nrm1f[:, j, :],
                            op0=ALU.mult,
                            op1=ALU.add,
                        )
                    else:
                        nc.vector.tensor_scalar(
                            out=fft, in0=ps[:, :],
                            scalar1=RS, scalar2=c_sb[:, B2E + j: B2E + j + 1],
                            op0=ALU.mult, op1=ALU.add,
                        )
                        nc.vector.scalar_tensor_tensor(
                            out=r2T[:, j, :],
                            in0=nrm1f[:, j, :],
                            scalar=c_sb[:, G1 + j: G1 + j + 1],
                            in1=fft,
                            op0=ALU.mult,
                            op1=ALU.add,
                        )

                def apply_ln2(jj, cen, rstd, sl):
                    # out = (cen * gamma2) * rstd; beta2 is added on the host
                    ot = tmp.tile([128, S], F32, tag="ot", bufs=4, name="ot")
                    if simple:
                        nc.vector.tensor_tensor(
                            out=ot[:, sl], in0=cen[:, sl], in1=rstd[:, sl],
                            op=ALU.mult,
                        )
                    else:
                        nc.vector.scalar_tensor_tensor(
                            out=ot[:, sl], in0=cen[:, sl],
                            scalar=c_sb[:, G2 + jj: G2 + jj + 1],
                            in1=rstd[:, sl], op0=ALU.mult, op1=ALU.mult,
                        )
                    nc.sync.dma_start(out=out_d[ts(jj, 128), sl], in_=ot[:, sl])

                layer_norm(tc, nc, psD, tmp, ones_sum, eps_sb, r2T, apply_ln2)

    nc.finalize()
    return nc


_NC_CACHE = {}


def _get_nc(simple):
    if simple not in _NC_CACHE:
        _NC_CACHE[simple] = build_nc(simple=simple)
    return _NC_CACHE[simple]


def make_in_maps(hidden_states, attention_mask, Wq, bq, Wk, bk, Wv, bv, Wo, bo,
                 ln1_g, ln1_b, W1, b1, W2, b2, ln2_g, ln2_b):
    """Host-side sharding + layout/dtype prep. Returns one map per core."""
    f32 = np.float32
    fp8 = ml_dtypes.float8_e4m3fn
    Wq, Wk, Wv, Wo = (np.asarray(w, f32) for w in (Wq, Wk, Wv, Wo))
    W1, W2 = np.asarray(W1, f32), np.asarray(W2, f32)
    g1 = np.asarray(ln1_g, f32)
    b1f = np.asarray(b1, f32) + np.asarray(ln1_b, f32) @ W1  # b1 + W1^T beta1
    W1g = W1 * g1[:, None]                                   # fold gamma1
    bo_eff = np.asarray(bo, f32) + np.asarray(bv, f32) @ Wo

    def q8(w):
        return (w * SW).astype(fp8)


    wqb = np.ascontiguousarray(
        q8(Wq).reshape(CH, 128, CH, 128).transpose(2, 1, 0, 3))
    wkb = np.ascontiguousarray(
        q8(Wk).reshape(CH, 128, CH, 128).transpose(2, 1, 0, 3))
    def q8hl(w):
        """Stack hi + same-scale residual planes along the contraction dim."""
        hi = q8(w)
        lo = q8(w - hi.astype(f32) / SW)
        return np.concatenate([hi, lo], axis=0)  # [2*K, N]

    w1hl = q8hl(W1g)   # [2*H, FF]
    w1b = np.ascontiguousarray(
        w1hl.reshape(2 * CH, 128, CF, 128).transpose(2, 1, 0, 3))
    w2hl = q8hl(W2)    # [2*FF, H]
    w2b = np.ascontiguousarray(
        w2hl.reshape(2 * CF, 128, CH, 128).transpose(2, 1, 0, 3))

    def cols(v, n):
        return np.ascontiguousarray(np.asarray(v, f32).reshape(n, 128).T)

    maskb = (1.0 - np.asarray(attention_mask, f32)) * -10000.0  # [B, S]

    base = np.zeros((128, NCONST), f32)
    base[:, BQ:BQ + CH] = cols(bq, CH)
    base[:, BK:BK + CH] = cols(bk, CH)
    base[:, G1:G1 + CH] = cols(g1, CH)
    base[:, B2E:B2E + CH] = cols(np.asarray(b2, f32) + np.asarray(ln1_b, f32),
                                 CH)
    base[:, G2:G2 + CH] = cols(ln2_g, CH)
    base[:, B2O:B2O + CH] = cols(ln2_b, CH)
    base[:, B1:B1 + CF] = cols(b1f, CF)

    x = np.asarray(hidden_states, f32)
    in_maps = []
    for b in range(B):
        consts = base.copy()
        consts[:, MB:MB + T] = cols(maskb[b], T)
        xT = np.ascontiguousarray(x[b].T)
        in_maps.append({
            "xq": xT.astype(fp8),
            "xr": xT + bo_eff[:, None],
            "wqb": wqb, "wkb": wkb,
            "wv": q8(Wv), "wo": q8(Wo),
            "w1b": w1b, "w2b": w2b,
            "consts": consts,
        })
    return in_maps


def kernel(**inputs):
    z = lambda *ks: all(
        np.all(np.asarray(inputs[k], np.float32) == 0.0) for k in ks)
    o = lambda *ks: all(
        np.all(np.asarray(inputs[k], np.float32) == 1.0) for k in ks)
    simple = bool(
        z("bq", "bk", "b1", "b2", "ln1_b") and o("ln1_g", "ln2_g")
        and np.all(np.asarray(inputs["attention_mask"]) == 1))
    nc = _get_nc(simple)
    in_maps = make_in_maps(**inputs)
    res = run_bass_kernel_spmd(nc, in_maps, core_ids=list(range(B)))
    out = np.stack([np.ascontiguousarray(r["outT"].T) for r in res.results])
    # beta2 is applied host-side (LN2's affine offset commutes with gather)
    out = out + np.asarray(inputs["ln2_b"], np.float32)
    return out.astype(np.float32)


# revision 15
# speedup vs baseline: 2.4228x; 2.4228x over previous
"""BERT layer (B=8, S=512, H=768, NH=12, DH=64, FF=3072) on 8 Trainium2 cores.

Data-parallel over batch (1 element/core, no collectives), feature-major
on-chip layout (activations as X^T [feat partitions, tokens free]), and
fp8e4m3 DoubleRow matmuls (2 k-planes per instruction, 0.5 PE cycles/row)
for every large GEMM:

  Q^T/K^T : fp8-DR (Wq,Wk x64 fp8; x fp8) -> bf16 via DVE descale(+bias)
  V       : fp8-DR (token-major), DVE descale -> v_q fp8
  S^T     : bf16 (K on partitions, per-head, quadrant-packed)
  P^T     : exp on ACT (scale=1/8 folded, mask bias) -> fp8
  den     : fp8-DR ones(=1/64) @ P^T -> broadcast rows; recip -> x64 ctx scale
  ctx^T   : fp8-DR (v_q @ P^T), DVE normalize -> ctx fp8 (x64 scaled)
  attn    : fp8-DR (Wo x64), descale 1/(64*64) + residual via DVE stt
            (bo + bv@Wo folded into the residual copy of x on host)
  LN1     : stats via f32r ones-matmuls; gamma/beta folded into W1/b1 (host),
            so FFN1 consumes raw (r1-mu)*rstd as fp8
  FFN1    : fp8-DR -> gelu on ACT (scale 1/64, bias b1') -> fp8
  FFN2    : fp8-DR -> descale + (b2+beta1) via DVE tensor_scalar,
            r2 = gamma1*nrm1 + ff via DVE stt
  LN2     : stats matmuls; output = gamma2*nrm2 + beta2 via tensor_scalar
"""

from contextlib import ExitStack

import numpy as np
import ml_dtypes

import concourse.bass as bass
from concourse import bacc
import concourse.tile as tile
from concourse import mybir
from concourse.bass_utils import run_bass_kernel_spmd

F32 = mybir.dt.float32
F32R = mybir.dt.float32r
BF16 = mybir.dt.bfloat16
FP8 = mybir.dt.float8e4
AF = mybir.ActivationFunctionType
ALU = mybir.AluOpType
DR = mybir.MatmulPerfMode.DoubleRow

B, S, H, NH, DH, FF = 8, 512, 768, 12, 64, 3072
EPS = 1e-3
CH = H // 128    # 6 hidden chunks
CU = CH // 2     # 3 DR chunk-pairs
CF = FF // 128   # 24 ff chunks
FU = CF // 2     # 12 DR pairs
T = S // 128     # 4 token/key chunks
NP = NH // 2     # 6 head pairs
SW = 64.0        # fp8 weight scale
RS = 1.0 / SW

# consts tile column map: [128, NCONST]
BQ, BK, G1, B2E, G2, B2O = 0, 6, 12, 18, 24, 30
B1 = 36          # 24 cols (b1 + W1^T beta1)
MB = B1 + CF     # 4 cols of mask bias
NCONST = MB + T


def ts(i, n):
    return slice(i * n, (i + 1) * n)


def build_nc(gelu_mode="hw", repeats=1, simple=True):
    nc = bacc.Bacc("TRN2", target_bir_lowering=False, debug=False)

    xq_d = nc.declare_dram_parameter("xq", [H, S], FP8, isOutput=False)
    xr_d = nc.declare_dram_parameter("xr", [H, S], F32, isOutput=False)
    wqb_d = nc.declare_dram_parameter("wqb", [CH, 128, CH, 128], FP8,
                                      isOutput=False)
    wkb_d = nc.declare_dram_parameter("wkb", [CH, 128, CH, 128], FP8,
                                      isOutput=False)
    wv_d = nc.declare_dram_parameter("wv", [H, H], FP8, isOutput=False)
    wo_d = nc.declare_dram_parameter("wo", [H, H], FP8, isOutput=False)
    w1_d = nc.declare_dram_parameter("w1b", [CF, 128, 2 * CH, 128], FP8,
                                     isOutput=False)
    w2_d = nc.declare_dram_parameter("w2b", [CH, 128, 2 * CF, 128], FP8,
                                     isOutput=False)
    c_d = nc.declare_dram_parameter("consts", [128, NCONST], F32, isOutput=False)
    out_d = nc.declare_dram_parameter("outT", [H, S], F32, isOutput=True)

    # feature-major DRAM views: (p, i, n) = W[i*128+p, n]
    def fmaj(d):
        return d.rearrange("(i p) n -> p i n", p=128)

    def layer_norm(tc, nc, pspool, tmp, ones_sum, eps_sb, src, apply_out):
        """Normalize src [128, CH, S] over features.

        Calls apply_out(jj, cen, rstd) per chunk — caller fuses the final
        (cen * rstd) multiply with whatever comes next.
        """
        sum_ps = pspool.tile([128, S], F32, tag="lnsum", bufs=1)
        sq_ps = pspool.tile([128, S], F32, tag="lnsq", bufs=1)
        for i in range(CH):
            nc.tensor.matmul(
                sum_ps[:, :], ones_sum[:, :], src[:, i, :],
                start=(i == 0), stop=(i == CH - 1),
            )
        for i in range(CH):
            sq = tmp.tile([128, S], F32R, tag="sq", bufs=2, name="sq")
            nc.scalar.activation(out=sq, in_=src[:, i, :], func=AF.Square)
            nc.tensor.matmul(
                sq_ps[:, :], ones_sum[:, :], sq,
                start=(i == 0), stop=(i == CH - 1),
            )
        mean = tmp.tile([128, S], F32, tag="mean", bufs=1, name="mean")
        nc.vector.tensor_scalar_mul(mean, sum_ps[:, :], 1.0 / H)
        negm2 = tmp.tile([128, S], F32, tag="negm2", bufs=1, name="negm2")
        nc.vector.scalar_tensor_tensor(
            out=negm2, in0=mean, scalar=-1.0, in1=mean, op0=ALU.mult, op1=ALU.mult
        )
        var = tmp.tile([128, S], F32, tag="var", bufs=1, name="var")
        nc.vector.scalar_tensor_tensor(
            out=var, in0=sq_ps[:, :], scalar=1.0 / H, in1=negm2,
            op0=ALU.mult, op1=ALU.add,
        )
        sd = tmp.tile([128, S], F32, tag="sd", bufs=1, name="sd")
        nc.scalar.activation(out=sd, in_=var, func=AF.Sqrt, bias=eps_sb[:, :])
        rstd = tmp.tile([128, S], F32, tag="rstd", bufs=1, name="rstd")
        nc.vector.reciprocal_approx_fast(out=rstd, in_=sd)
        for jj in range(CH):
            cen = tmp.tile([128, S], F32, tag="cen", bufs=2, name="cen")
            nc.vector.tensor_tensor(
                out=cen, in0=src[:, jj, :], in1=mean, op=ALU.subtract
            )
            apply_out(jj, cen, rstd)

    with tile.TileContext(nc) as tc, ExitStack() as top:
        cpool = top.enter_context(tc.tile_pool(name="cpool", bufs=1))
        c_sb = cpool.tile([128, NCONST], F32, name="c_sb")
        nc.sync.dma_start(out=c_sb, in_=c_d[:, :])
        ones8 = cpool.tile([128, 2, 64], FP8, name="ones8")
        nc.vector.memset(ones8, RS)  # 1/64: folds the x64 ctx scale into den
        ones_f32 = cpool.tile([128, 128], F32, name="ones_f32")
        nc.vector.memset(ones_f32, 1.0)
        ones_sum = cpool.tile([128, 128], F32R, name="ones_sum")
        nc.vector.tensor_copy(out=ones_sum, in_=ones_f32)
        eps_sb = cpool.tile([128, 1], F32, name="eps_sb")
        nc.vector.memset(eps_sb, EPS)

        mid = top.enter_context(tc.tile_pool(name="mid", bufs=1))
        tmp = top.enter_context(tc.tile_pool(name="tmp", bufs=1))

        for _rep in range(repeats):
            with ExitStack() as s_ac:
                apool = s_ac.enter_context(tc.tile_pool(name="apool", bufs=1))
                xq = apool.tile([128, CH, S], FP8, name="xq")
                nc.sync.dma_start(out=xq, in_=fmaj(xq_d))
                xr = apool.tile([128, CH, S], F32, name="xr")
                for _i in range(CH):
                    nc.sync.dma_start(out=xr[:, _i, :], in_=fmaj(xr_d)[:, _i, :])
                qT = apool.tile([128, CH, S], BF16, name="qT")
                kT = apool.tile([128, CH, S], BF16, name="kT")
                v_q = apool.tile([128, T, NH, DH], FP8, name="v_q")
                ctxq = apool.tile([128, CH, S], FP8, name="ctxq")

                wopool = s_ac.enter_context(tc.tile_pool(name="wopool", bufs=1))
                wo_sb = wopool.tile([128, CH, H], FP8, name="wo_sb")

                # ---- Fused phase A+B: V first, then per head pair:
                # project Q_j/K_j -> S^T -> exp -> denom -> ctx.
                with ExitStack() as s_a:
                    wqkv = s_a.enter_context(tc.tile_pool(name="wqkv", bufs=1))
                    wv_sb = wqkv.tile([128, CH, H], FP8, name="wv_sb")
                    nc.sync.dma_start(out=wv_sb, in_=fmaj(wv_d))
                    wqkpool = s_a.enter_context(
                        tc.tile_pool(name="wqkpool", bufs=3)
                    )
                    psA = s_a.enter_context(
                        tc.tile_pool(name="psA", bufs=1, space="PSUM")
                    )
                    bpool = s_a.enter_context(tc.tile_pool(name="bpool", bufs=1))

                    def project_qk(j):
                        for blk_d, dest, bcol in ((wqb_d, qT, BQ), (wkb_d, kT, BK)):
                            wt = wqkpool.tile([128, CH, 128], FP8, tag="wqk",
                                              name="wt")
                            nc.sync.dma_start(out=wt, in_=blk_d[j])
                            ps = psA.tile([128, S], F32, tag="mm", bufs=2,
                                          name="psqk")
                            for u in range(CU):
                                nc.tensor.matmul(
                                    ps[:, :],
                                    wt[:, ts(u, 2), :],
                                    xq[:, ts(u, 2), :],
                                    start=(u == 0),
                                    stop=(u == CU - 1),
                                    perf_mode=DR,
                                )
                            # (ps * 1/SW) + bias  -> bf16
                            if simple:
                                nc.vector.tensor_scalar_mul(
                                    dest[:, j, :], ps[:, :], RS
                                )
                            else:
                                nc.vector.tensor_scalar(
                                    out=dest[:, j, :], in0=ps[:, :],
                                    scalar1=RS,
                                    scalar2=c_sb[:, bcol + j: bcol + j + 1],
                                    op0=ALU.mult, op1=ALU.add,
                                )

                    project_qk(0)
                    for t in range(T):
                        for half in range(2):
                            psf = psA.tile([128, S], F32, tag="mm", bufs=2,
                                           name="psv")
                            ps = psf[:, 0:384]
                            for u in range(CU):
                                nc.tensor.matmul(
                                    ps[:, :],
                                    xq[:, ts(u, 2), ts(t, 128)],
                                    wv_sb[:, ts(u, 2), ts(half, 384)],
                                    start=(u == 0),
                                    stop=(u == CU - 1),
                                    perf_mode=DR,
                                )
                            nc.vector.tensor_scalar_mul(
                                v_q[:, t, ts(half, 6), :].rearrange(
                                    "p h d -> p (h d)"
                                ),
                                ps[:, :],
                                RS,
                            )

                    for j in range(NP):
                        if j > 0:
                            project_qk(j)
                        if j == 2:
                            nc.sync.dma_start(out=wo_sb, in_=fmaj(wo_d))
                        es = bpool.tile([128, T, S], FP8, tag="es", bufs=2,
                                        name="es")
                        esb = bpool.tile([128, T, S], FP8, tag="esb", bufs=2,
                                         name="esb")
                        for t in range(T):
                            for half, e in ((0, es), (1, esb)):
                                ps = psA.tile([128, S], F32, tag="sT", bufs=2,
                                              name="ps_sT")
                                nc.tensor.matmul(
                                    ps[:, :],
                                    kT[ts(half, 64), j, ts(t, 128)],
                                    qT[ts(half, 64), j, :],
                                    start=True,
                                    stop=True,
                                    tile_position=(half * 64, 0),
                                )
                                nc.scalar.activation(
                                    out=e[:, t, :],
                                    in_=ps[:, :],
                                    func=AF.Exp,
                                    bias=(0.0 if simple
                                          else c_sb[:, MB + t: MB + t + 1]),
                                    scale=0.125,
                                )
                        # per half: denominator rows (x 1/64), recip, ctx^T,
                        # normalize into ctxq (DR matmuls need dst base 0;
                        # the half-b partition shift happens in the DVE write)
                        for half, e in ((0, es), (1, esb)):
                            den_ps = psA.tile([128, S], F32, tag="den",
                                              bufs=1, name="den_ps")
                            for u in range(T // 2):
                                nc.tensor.matmul(
                                    den_ps[0:64, :],
                                    ones8[:, :, :],
                                    e[:, ts(u, 2), :],
                                    start=(u == 0),
                                    stop=(u == T // 2 - 1),
                                    perf_mode=DR,
                                )
                            recip = bpool.tile([128, S], F32, tag="recip",
                                               bufs=2, name="recip")
                            nc.vector.reciprocal_approx_fast(
                                out=recip[0:64, :], in_=den_ps[0:64, :]
                            )
                            ctx_ps = psA.tile([128, S], F32, tag="ctx",
                                              bufs=2, name="ctx_ps")
                            for u in range(T // 2):
                                nc.tensor.matmul(
                                    ctx_ps[0:64, :],
                                    v_q[:, ts(u, 2), 2 * j + half, :],
                                    e[:, ts(u, 2), :],
                                    start=(u == 0),
                                    stop=(u == T // 2 - 1),
                                    perf_mode=DR,
                                )
                            nc.vector.tensor_tensor(
                                out=ctxq[ts(half, 64), j, :],
                                in0=ctx_ps[0:64, :],
                                in1=recip[0:64, :],
                                op=ALU.mult,
                            )

                # ---- Phase C: Wo^T @ ctx^T (fp8-DR), +residual (xr), LN1 ----
                r1T = mid.tile([128, CH, S], F32R, name="r1T")
                nrm1q = mid.tile([128, CH, S], FP8, name="nrm1q")
                nrm1f = mid.tile([128, CH, S], F32, name="nrm1f")
                with ExitStack() as s_c:
                    psC = s_c.enter_context(
                        tc.tile_pool(name="psC", bufs=1, space="PSUM")
                    )
                    for j in range(CH):
                        ps = psC.tile([128, S], F32, tag="mm", bufs=3, name="ps_wo")
                        for u in range(CU):
                            nc.tensor.matmul(
                                ps[:, :],
                                wo_sb[:, ts(u, 2), ts(j, 128)],
                                ctxq[:, ts(u, 2), :],
                                start=(u == 0),
                                stop=(u == CU - 1),
                                perf_mode=DR,
                            )
                        # r1 = ps/(SW*SW) + xr   (bo+bv@Wo folded into xr)
                        nc.vector.scalar_tensor_tensor(
                            out=r1T[:, j, :],
                            in0=ps[:, :],
                            scalar=RS * RS,
                            in1=xr[:, j, :],
                            op0=ALU.mult,
                            op1=ALU.add,
                        )

                    def apply_ln1(jj, cen, rstd):
                        # gamma1/beta1 folded into W1/b1; keep raw nrm (f32)
                        # for the r2 residual and fp8 for FFN1.
                        nc.vector.tensor_tensor(
                            out=nrm1f[:, jj, :], in0=cen, in1=rstd, op=ALU.mult
                        )
                        nc.vector.tensor_copy(
                            out=nrm1q[:, jj, :], in_=nrm1f[:, jj, :]
                        )

                    layer_norm(tc, nc, psC, tmp, ones_sum, eps_sb, r1T, apply_ln1)

            # ---- Phase D: ff1 = gelu(W1'^T @ nrm1 + b1'), fp8-DR ----
            with ExitStack() as s_de:
                fpool = s_de.enter_context(tc.tile_pool(name="fpool", bufs=1))
                geluT = fpool.tile([128, CF, S], FP8, name="geluT")
                w1pool = s_de.enter_context(tc.tile_pool(name="w1pool", bufs=12))
                psD = s_de.enter_context(tc.tile_pool(name="psD", bufs=1,
                                                      space="PSUM"))
                for f in range(CF):
                    w1t = w1pool.tile([128, 2 * CH, 128], FP8, tag="w1",
                                      name="w1t")
                    nc.sync.dma_start(out=w1t, in_=w1_d[f])
                    ps = psD.tile([128, S], F32, tag="mm", bufs=4, name="ps_f1")
                    for u in range(2 * CU):  # hi then lo weight planes
                        nc.tensor.matmul(
                            ps[:, :],
                            w1t[:, ts(u, 2), :],
                            nrm1q[:, ts(u % CU, 2), :],
                            start=(u == 0),
                            stop=(u == 2 * CU - 1),
                            perf_mode=DR,
                        )
                    nc.scalar.activation(
                        out=geluT[:, f, :],
                        in_=ps[:, :],
                        func=AF.Gelu,
                        bias=(0.0 if simple else c_sb[:, B1 + f: B1 + f + 1]),
                        scale=RS,
                    )

                # ---- Phase E: ff2 (fp8-DR), r2 = g1*nrm1 + ff + (b2+beta1),
                # LN2 ----
                r2T = mid.tile([128, CH, S], F32R, name="r2T")
                w2pool = s_de.enter_context(tc.tile_pool(name="w2pool", bufs=6))
                for j in range(CH):
                    w2t = w2pool.tile([128, 2 * CF, 128], FP8, tag="w2",
                                      name="w2t")
                    nc.sync.dma_start(out=w2t, in_=w2_d[j])
                    ps = psD.tile([128, S], F32, tag="mm", bufs=4, name="ps_f2")
                    for u in range(2 * FU):  # hi then lo weight planes
                        nc.tensor.matmul(
                            ps[:, :],
                            w2t[:, ts(u, 2), :],
                            geluT[:, ts(u % FU, 2), :],
                            start=(u == 0),
                            stop=(u == 2 * FU - 1),
                            perf_mode=DR,
                        )
                    fft = tmp.tile([128, S], F32, tag="fft", bufs=2, name="fft")
                    if simple:
                        # r2 = ps/SW + nrm1 (gamma1==1, b2+beta1==0)
                        nc.vector.scalar_tensor_tensor(
                            out=r2T[:, j, :],
                            in0=ps[:, :],
                            scalar=RS,
                            in1=nrm1f[:, j, :],
                            op0=ALU.mult,
                            op1=ALU.add,
                        )
                    else:
                        nc.vector.tensor_scalar(
                            out=fft, in0=ps[:, :],
                            scalar1=RS, scalar2=c_sb[:, B2E + j: B2E + j + 1],
                            op0=ALU.mult, op1=ALU.add,
                        )
                        nc.vector.scalar_tensor_tensor(
                            out=r2T[:, j, :],
                            in0=nrm1f[:, j, :],
                            scalar=c_sb[:, G1 + j: G1 + j + 1],
                            in1=fft,
                            op0=ALU.mult,
                            op1=ALU.add,
                        )

                def apply_ln2(jj, cen, rstd):
                    # out = (cen * gamma2) * rstd; beta2 is added on the host
                    ot = tmp.tile([128, S], F32, tag="ot", bufs=2, name="ot")
                    if simple:
                        nc.vector.tensor_tensor(
                            out=ot, in0=cen, in1=rstd, op=ALU.mult
                        )
                    else:
                        nc.vector.scalar_tensor_tensor(
                            out=ot, in0=cen,
                            scalar=c_sb[:, G2 + jj: G2 + jj + 1],
                            in1=rstd, op0=ALU.mult, op1=ALU.mult,
                        )
                    nc.sync.dma_start(out=out_d[ts(jj, 128), :], in_=ot)

                layer_norm(tc, nc, psD, tmp, ones_sum, eps_sb, r2T, apply_ln2)

    nc.finalize()
    return nc


_NC_CACHE = {}


def _get_nc(simple):
    if simple not in _NC_CACHE:
        _NC_CACHE[simple] = build_nc(simple=simple)
    return _NC_CACHE[simple]


def make_in_maps(hidden_states, attention_mask, Wq, bq, Wk, bk, Wv, bv, Wo, bo,
                 ln1_g, ln1_b, W1, b1, W2, b2, ln2_g, ln2_b):
    """Host-side sharding + layout/dtype prep. Returns one map per core."""
    f32 = np.float32
    fp8 = ml_dtypes.float8_e4m3fn
    Wq, Wk, Wv, Wo = (np.asarray(w, f32) for w in (Wq, Wk, Wv, Wo))
    W1, W2 = np.asarray(W1, f32), np.asarray(W2, f32)
    g1 = np.asarray(ln1_g, f32)
    b1f = np.asarray(b1, f32) + np.asarray(ln1_b, f32) @ W1  # b1 + W1^T beta1
    W1g = W1 * g1[:, None]                                   # fold gamma1
    bo_eff = np.asarray(bo, f32) + np.asarray(bv, f32) @ Wo

    def q8(w):
        return (w * SW).astype(fp8)


    wqb = np.ascontiguousarray(
        q8(Wq).reshape(CH, 128, CH, 128).transpose(2, 1, 0, 3))
    wkb = np.ascontiguousarray(
        q8(Wk).reshape(CH, 128, CH, 128).transpose(2, 1, 0, 3))
    def q8hl(w):
        """Stack hi + same-scale residual planes along the contraction dim."""
        hi = q8(w)
        lo = q8(w - hi.astype(f32) / SW)
        return np.concatenate([hi, lo], axis=0)  # [2*K, N]

    w1hl = q8hl(W1g)   # [2*H, FF]
    w1b = np.ascontiguousarray(
        w1hl.reshape(2 * CH, 128, CF, 128).transpose(2, 1, 0, 3))
    w2hl = q8hl(W2)    # [2*FF, H]
    w2b = np.ascontiguousarray(
        w2hl.reshape(2 * CF, 128, CH, 128).transpose(2, 1, 0, 3))

    def cols(v, n):
        return np.ascontiguousarray(np.asarray(v, f32).reshape(n, 128).T)

    maskb = (1.0 - np.asarray(attention_mask, f32)) * -10000.0  # [B, S]

    base = np.zeros((128, NCONST), f32)
    base[:, BQ:BQ + CH] = cols(bq, CH)
    base[:, BK:BK + CH] = cols(bk, CH)
    base[:, G1:G1 + CH] = cols(g1, CH)
    base[:, B2E:B2E + CH] = cols(np.asarray(b2, f32) + np.asarray(ln1_b, f32),
                                 CH)
    base[:, G2:G2 + CH] = cols(ln2_g, CH)
    base[:, B2O:B2O + CH] = cols(ln2_b, CH)
    base[:, B1:B1 + CF] = cols(b1f, CF)

    x = np.asarray(hidden_states, f32)
    in_maps = []
    for b in range(B):
        consts = base.copy()
        consts[:, MB:MB + T] = cols(maskb[b], T)
        xT = np.ascontiguousarray(x[b].T)
        in_maps.append({
            "xq": xT.astype(fp8),
            "xr": xT + bo_eff[:, None],
            "wqb": wqb, "wkb": wkb,
            "wv": q8(Wv), "wo": q8(Wo),
            "w1b": w1b, "w2b": w2b,
            "consts": consts,
        })
    return in_maps


def kernel(**inputs):
    z = lambda *ks: all(
        np.all(np.asarray(inputs[k], np.float32) == 0.0) for k in ks)
    o = lambda *ks: all(
        np.all(np.asarray(inputs[k], np.float32) == 1.0) for k in ks)
    simple = bool(
        z("bq", "bk", "b1", "b2", "ln1_b") and o("ln1_g", "ln2_g")
        and np.all(np.asarray(inputs["attention_mask"]) == 1))
    nc = _get_nc(simple)
    in_maps = make_in_maps(**inputs)
    res = run_bass_kernel_spmd(nc, in_maps, core_ids=list(range(B)))
    out = np.stack([np.ascontiguousarray(r["outT"].T) for r in res.results])
    # beta2 is applied host-side (LN2's affine offset commutes with gather)
    out = out + np.asarray(inputs["ln2_b"], np.float32)
    return out.astype(np.float32)
